# revision 8
# baseline (speedup 1.0000x reference)
"""Local (windowed) self-attention Trainium2 kernel.

Model (reference): LayerNorm -> per-window (W=1024) multi-head attention
(H=8 heads, K=32 head dim) -> output projection -> residual add.
Shapes: x [B=2, T=8192, C=512]; 16 independent windows of 1024 tokens.

Distribution: 16 windows / 8 cores = 2 windows per core (data parallel over
the B*n_chunks axis), QKV/O weights replicated, no collectives.

Per-core program (all loops statically unrolled, Tile framework):
  Phase 1 (both windows): LayerNorm in [tok, C] layout via bn_stats,
    normalize + cast to bf16, bounce z through DRAM with DMA-transpose to
    build zT [C, tok] (contraction layouts for the projections).
  Phase 2 (per window):
    - QT, KT [hd, tok] and V [tok, hd] projections (bf16 matmuls, C
      contraction over 4 chunks of 128).
    - Attention per (q-tile of 512, head-group of 4): scores computed
      TRANSPOSED scoresT [s-chunk 128, q 512] with 4-way PE row tiling
      (head dim 32 -> tile_position=(32g, 0)); one ScalarE Exp op over the
      4 accumulated PSUM banks [128, 2048] -> bf16 expT in SBUF;
      A·V and the softmax denominator (ones-matmul) accumulate over the 8
      s-chunks with 4-way col tiling (tile_position=(0, 32g)).
    - Normalize oT by reciprocal of denominator (VectorE), out proj
      (contraction hd=256) + bias rank-1 matmul + residual add (fp32).

Host-side prep: shard windows, fold LN gamma/beta into the projection
weights/biases, fold bv through the attention (softmax rows sum to 1) into
the output bias, cast weights to bf16.
"""

import numpy as np
import ml_dtypes

import concourse.bass as bass
import concourse.tile as tile
from concourse import bacc, mybir
from concourse.bass_utils import run_bass_kernel_spmd

F32 = mybir.dt.float32
BF16 = mybir.dt.bfloat16

B, T, C, H, K = 2, 8192, 512, 8, 32
W = 1024
HK = H * K              # 256
N_CORES = 8
NW = (B * T) // W       # 16 windows
WPC = NW // N_CORES     # 2 windows per core
EPS = 1e-5
SCALE = 1.0 / np.sqrt(K)

TOK_TILES = W // 128    # 8 token tiles per window
C_CHUNKS = C // 128     # 4
HD_TILES = HK // 128    # 2
Q_TILES = W // 512      # 2 query tiles per window
S_CHUNKS = W // 128     # 8 key chunks per window


def _build_program():
    nc = bacc.Bacc("TRN2", target_bir_lowering=False, debug=False)

    x_d = nc.dram_tensor("x", [WPC * W, C], F32, kind="ExternalInput")
    wq_d = nc.dram_tensor("wq", [C_CHUNKS, 128, HK], BF16, kind="ExternalInput")
    wk_d = nc.dram_tensor("wk", [C_CHUNKS, 128, HK], BF16, kind="ExternalInput")
    wv_d = nc.dram_tensor("wv", [C_CHUNKS, 128, HK], BF16, kind="ExternalInput")
    wo_d = nc.dram_tensor("wo", [HD_TILES, 128, C], BF16, kind="ExternalInput")
    bq_d = nc.dram_tensor("bq", [HD_TILES, 128, 1], F32, kind="ExternalInput")
    bk_d = nc.dram_tensor("bk", [HD_TILES, 128, 1], F32, kind="ExternalInput")
    bo_d = nc.dram_tensor("bo", [1, C], BF16, kind="ExternalInput")
    out_d = nc.dram_tensor("out", [WPC * W, C], F32, kind="ExternalOutput")
    z_d = nc.dram_tensor("z_scratch", [WPC, W, C], BF16)

    with tile.TileContext(nc) as tc:
        with (
            tc.tile_pool(name="const", bufs=1) as const,
            tc.tile_pool(name="xres", bufs=1) as xres,
            tc.tile_pool(name="zt", bufs=1) as ztp,
            tc.tile_pool(name="ln", bufs=4) as ln,
            tc.tile_pool(name="zw", bufs=3) as zw,
            tc.tile_pool(name="qk", bufs=2) as qk,
            tc.tile_pool(name="vp", bufs=2) as vp,
            tc.tile_pool(name="ot", bufs=2) as otp,
            tc.tile_pool(name="ex", bufs=2) as ex,
            tc.tile_pool(name="tmp", bufs=4) as tmp,
            tc.tile_pool(name="outp", bufs=3) as outp,
            tc.tile_pool(name="ps_proj", bufs=2, space="PSUM") as ps_proj,
            tc.tile_pool(name="ps_sc", bufs=1, space="PSUM") as ps_sc_pool,
            tc.tile_pool(name="ps_acc", bufs=1, space="PSUM") as ps_acc,
        ):
            # ---- constants / weights -------------------------------------
            ones32 = const.tile([128, 32], BF16)
            nc.vector.memset(ones32, 1.0)
            ones1 = const.tile([1, 128], BF16)
            nc.vector.memset(ones1, 1.0)
            eps_t = const.tile([128, 1], F32)
            nc.vector.memset(eps_t, EPS)

            wq_s = const.tile([128, C_CHUNKS, HK], BF16)
            wk_s = const.tile([128, C_CHUNKS, HK], BF16)
            wv_s = const.tile([128, C_CHUNKS, HK], BF16)
            for c in range(C_CHUNKS):
                nc.sync.dma_start(wq_s[:, c, :], wq_d[c])
                nc.sync.dma_start(wk_s[:, c, :], wk_d[c])
                nc.sync.dma_start(wv_s[:, c, :], wv_d[c])
            wo_s = const.tile([128, HD_TILES, C], BF16)
            for g in range(HD_TILES):
                nc.sync.dma_start(wo_s[:, g, :], wo_d[g])
            bq_s = const.tile([128, HD_TILES], F32)
            bk_s = const.tile([128, HD_TILES], F32)
            for m in range(HD_TILES):
                nc.sync.dma_start(bq_s[:, m : m + 1], bq_d[m])
                nc.sync.dma_start(bk_s[:, m : m + 1], bk_d[m])
            bo_s = const.tile([1, C], BF16)
            nc.sync.dma_start(bo_s, bo_d[:])

            # ---- phase 1: LayerNorm + transpose bounce (both windows) ----
            xs = [
                [xres.tile([128, C], F32, name=f"x_{w}_{t}", tag=f"x_{w}_{t}") for t in range(TOK_TILES)]
                for w in range(WPC)
            ]
            zT = [
                [ztp.tile([128, W], BF16, name=f"zT_{w}_{c}", tag=f"zT_{w}_{c}") for c in range(C_CHUNKS)]
                for w in range(WPC)
            ]
            for w in range(WPC):
                for t in range(TOK_TILES):
                    x_t = xs[w][t]
                    nc.sync.dma_start(x_t, x_d[(w * TOK_TILES + t) * 128 :][:128, :])
                    stats = ln.tile([128, 6], F32, tag="stats")
                    nc.vector.bn_stats(out=stats, in_=x_t)
                    mv = ln.tile([128, 2], F32, tag="mv")
                    nc.vector.bn_aggr(out=mv, in_=stats)
                    std = ln.tile([128, 1], F32, tag="std")
                    nc.scalar.activation(
                        out=std,
                        in_=mv[:, 1:2],
                        func=mybir.ActivationFunctionType.Sqrt,
                        bias=eps_t[:],
                    )
                    rstd = ln.tile([128, 1], F32, tag="std")
                    nc.vector.reciprocal(out=rstd, in_=std)
                    z_t = zw.tile([128, C], BF16, tag="z")
                    nc.vector.tensor_scalar(
                        out=z_t,
                        in0=x_t,
                        scalar1=mv[:, 0:1],
                        scalar2=rstd,
                        op0=mybir.AluOpType.subtract,
                        op1=mybir.AluOpType.mult,
                    )
                    nc.sync.dma_start(z_d[w, t * 128 :][:128, :], z_t)
                for c in range(C_CHUNKS):
                    nc.sync.dma_start(
                        zT[w][c], z_d[w][:, c * 128 : (c + 1) * 128], transpose=True
                    )

            # ---- phase 2: per-window attention ---------------------------
            for w in range(WPC):
                # --- QKV projections ---
                qkt = {}
                for name, w_s, b_s in (("q", wq_s, bq_s), ("k", wk_s, bk_s)):
                    for m in range(HD_TILES):
                        dst = qk.tile([128, W], BF16, name=f"{name}T_{m}", tag=f"{name}T_{m}")
                        qkt[(name, m)] = dst
                        for n in range(Q_TILES):
                            ps = ps_proj.tile([128, 512], F32, tag="proj")
                            for c in range(C_CHUNKS):
                                nc.tensor.matmul(
                                    ps,
                                    lhsT=w_s[:, c, m * 128 : (m + 1) * 128],
                                    rhs=zT[w][c][:, n * 512 : (n + 1) * 512],
                                    start=(c == 0),
                                    stop=(c == C_CHUNKS - 1),
                                )
                            nc.vector.tensor_scalar_add(
                                out=dst[:, n * 512 : (n + 1) * 512],
                                in0=ps,
                                scalar1=b_s[:, m : m + 1],
                            )
                v_s = vp.tile([128, TOK_TILES, HK], BF16, tag="v")
                for t in range(TOK_TILES):
                    ps = ps_proj.tile([128, 512], F32, tag="proj")
                    psv = ps[:, :HK]
                    for c in range(C_CHUNKS):
                        nc.tensor.matmul(
                            psv,
                            lhsT=zT[w][c][:, t * 128 : (t + 1) * 128],
                            rhs=wv_s[:, c, :],
                            start=(c == 0),
                            stop=(c == C_CHUNKS - 1),
                        )
                    nc.vector.tensor_copy(v_s[:, t, :], psv)

                # --- attention ---
                oT = [otp.tile([128, W], BF16, name=f"oT_{g}", tag=f"oT_{g}") for g in range(HD_TILES)]
                for qt in range(Q_TILES):
                    for hg in range(HD_TILES):
                        ps_oT = ps_acc.tile([128, 512], F32, tag="oT")
                        ps_den = ps_acc.tile([128, 512], F32, tag="den")
                        for cch in range(S_CHUNKS):
                            ps_sc = ps_sc_pool.tile([128, 2048], F32, tag="sc")
                            for g in range(4):
                                nc.tensor.matmul(
                                    ps_sc[:, g * 512 : (g + 1) * 512],
                                    lhsT=qkt[("k", hg)][
                                        g * 32 : (g + 1) * 32,
                                        cch * 128 : (cch + 1) * 128,
                                    ],
                                    rhs=qkt[("q", hg)][
                                        g * 32 : (g + 1) * 32,
                                        qt * 512 : (qt + 1) * 512,
                                    ],
                                    tile_position=(g * 32, 0),
                                )
                            expT = ex.tile([128, 2048], BF16, tag="exp")
                            nc.scalar.activation(
                                out=expT, in_=ps_sc,
                                func=mybir.ActivationFunctionType.Exp,
                                scale=float(SCALE),
                            )
                            for g in range(4):
                                h = 4 * hg + g
                                nc.tensor.matmul(
                                    ps_oT[g * 32 : (g + 1) * 32, :],
                                    lhsT=v_s[:, cch, h * 32 : (h + 1) * 32],
                                    rhs=expT[:, g * 512 : (g + 1) * 512],
                                    start=(cch == 0),
                                    stop=(cch == S_CHUNKS - 1),
                                    tile_position=(0, g * 32),
                                    skip_group_check=True,
                                )
                                nc.tensor.matmul(
                                    ps_den[g * 32 : (g + 1) * 32, :],
                                    lhsT=ones32[:],
                                    rhs=expT[:, g * 512 : (g + 1) * 512],
                                    start=(cch == 0),
                                    stop=(cch == S_CHUNKS - 1),
                                    tile_position=(0, g * 32),
                                    skip_group_check=True,
                                )
                        rec = tmp.tile([128, 512], F32, tag="rec")
                        nc.vector.reciprocal(out=rec, in_=ps_den)
                        nc.vector.tensor_mul(
                            out=oT[hg][:, qt * 512 : (qt + 1) * 512],
                            in0=ps_oT,
                            in1=rec,
                        )

                # --- output projection + residual ---
                for t in range(TOK_TILES):
                    ps = ps_proj.tile([128, 512], F32, tag="proj")
                    for g in range(HD_TILES):
                        nc.tensor.matmul(
                            ps,
                            lhsT=oT[g][:, t * 128 : (t + 1) * 128],
                            rhs=wo_s[:, g, :],
                            start=(g == 0),
                            stop=False,
                        )
                    nc.tensor.matmul(
                        ps, lhsT=ones1[:, :128], rhs=bo_s, start=False, stop=True
                    )
                    o_t = outp.tile([128, C], F32, tag="o")
                    nc.vector.tensor_add(out=o_t, in0=ps, in1=xs[w][t])
                    nc.sync.dma_start(out_d[(w * TOK_TILES + t) * 128 :][:128, :], o_t)

    nc.compile()
    return nc


_CACHE = {}


def _get_program():
    if "nc" not in _CACHE:
        _CACHE["nc"] = _build_program()
    return _CACHE["nc"]


def _prep_inputs(x, ln_gamma, ln_beta, Wq, bq, Wk, bk, Wv, bv, Wo, bo):
    """Host-side constant folding + sharding. Returns per-core in_maps."""
    x = np.asarray(x, np.float32)
    g = np.asarray(ln_gamma, np.float32)
    be = np.asarray(ln_beta, np.float32)
    Wq = np.asarray(Wq, np.float32).reshape(C, HK)
    Wk = np.asarray(Wk, np.float32).reshape(C, HK)
    Wv = np.asarray(Wv, np.float32).reshape(C, HK)
    Wo2 = np.asarray(Wo, np.float32).reshape(HK, C)
    bq = np.asarray(bq, np.float32).reshape(HK)
    bk = np.asarray(bk, np.float32).reshape(HK)
    bv = np.asarray(bv, np.float32).reshape(HK)
    bo = np.asarray(bo, np.float32).reshape(C)

    # Fold LN affine (z = n*gamma + beta) into projections:
    #   z @ W + b = n @ (gamma[:,None]*W) + (beta @ W + b)
    Wq_e = g[:, None] * Wq
    Wk_e = g[:, None] * Wk
    Wv_e = g[:, None] * Wv
    bq_e = be @ Wq + bq
    bk_e = be @ Wk + bk
    bv_e = be @ Wv + bv
    # Softmax rows sum to 1 -> value bias passes through attention:
    #   attn @ (V + 1 bv) @ Wo + bo = attn @ V @ Wo + (bv @ Wo + bo)
    bo_e = bv_e @ Wo2 + bo

    bf = ml_dtypes.bfloat16
    wq_h = Wq_e.reshape(C_CHUNKS, 128, HK).astype(bf)
    wk_h = Wk_e.reshape(C_CHUNKS, 128, HK).astype(bf)
    wv_h = Wv_e.reshape(C_CHUNKS, 128, HK).astype(bf)
    wo_h = Wo2.reshape(HD_TILES, 128, C).astype(bf)
    bq_h = bq_e.reshape(HD_TILES, 128, 1).astype(np.float32)
    bk_h = bk_e.reshape(HD_TILES, 128, 1).astype(np.float32)
    bo_h = bo_e.reshape(1, C).astype(bf)

    xw = np.ascontiguousarray(x.reshape(NW, W, C))
    in_maps = []
    for i in range(N_CORES):
        shard = np.ascontiguousarray(
            xw[i * WPC : (i + 1) * WPC].reshape(WPC * W, C)
        )
        in_maps.append(
            {
                "x": shard,
                "wq": wq_h, "wk": wk_h, "wv": wv_h, "wo": wo_h,
                "bq": bq_h, "bk": bk_h, "bo": bo_h,
            }
        )
    return in_maps


def kernel(x, ln_gamma, ln_beta, Wq, bq, Wk, bk, Wv, bv, Wo, bo):
    nc = _get_program()
    in_maps = _prep_inputs(x, ln_gamma, ln_beta, Wq, bq, Wk, bk, Wv, bv, Wo, bo)
    res = run_bass_kernel_spmd(nc, in_maps, core_ids=list(range(N_CORES)))
    out = np.concatenate([res.results[i]["out"] for i in range(N_CORES)], axis=0)
    return np.ascontiguousarray(out.reshape(B, T, C)).astype(np.float32)


# revision 9
# speedup vs baseline: 399.7549x; 399.7549x over previous
"""Local (windowed) self-attention Trainium2 kernel.

Model (reference): LayerNorm -> per-window (W=1024) multi-head attention
(H=8 heads, K=32 head dim) -> output projection -> residual add.
Shapes: x [B=2, T=8192, C=512]; 16 independent windows of 1024 tokens.

Distribution: 16 windows / 8 cores = 2 windows per core (data parallel over
the B*n_chunks axis), QKV/O weights replicated, no collectives.

Per-core program (all loops statically unrolled, Tile framework):
  Phase 1 (both windows): LayerNorm in [tok, C] layout via bn_stats,
    normalize + cast to bf16, bounce z through DRAM with DMA-transpose to
    build zT [C, tok] (contraction layouts for the projections).
  Phase 2 (per window):
    - QT, KT [hd, tok] and V [tok, hd] projections (bf16 matmuls, C
      contraction over 4 chunks of 128).
    - Attention per (q-tile of 512, head-group of 4): scores computed
      TRANSPOSED scoresT [s-chunk 128, q 512] with 4-way PE row tiling
      (head dim 32 -> tile_position=(32g, 0)); one ScalarE Exp op over the
      4 accumulated PSUM banks [128, 2048] -> bf16 expT in SBUF;
      A·V and the softmax denominator (ones-matmul) accumulate over the 8
      s-chunks with 4-way col tiling (tile_position=(0, 32g)).
    - Normalize oT by reciprocal of denominator (VectorE), out proj
      (contraction hd=256) + bias rank-1 matmul + residual add (fp32).

Host-side prep: shard windows, fold LN gamma/beta into the projection
weights/biases, fold bv through the attention (softmax rows sum to 1) into
the output bias, cast weights to bf16.
"""

import numpy as np
import ml_dtypes

import concourse.bass as bass
import concourse.tile as tile
from concourse import bacc, mybir
from concourse.bass_utils import run_bass_kernel_spmd

F32 = mybir.dt.float32
BF16 = mybir.dt.bfloat16

B, T, C, H, K = 2, 8192, 512, 8, 32
W = 1024
HK = H * K              # 256
N_CORES = 8
NW = (B * T) // W       # 16 windows
WPC = NW // N_CORES     # 2 windows per core
EPS = 1e-5
SCALE = 1.0 / np.sqrt(K)

TOK_TILES = W // 128    # 8 token tiles per window
C_CHUNKS = C // 128     # 4
HD_TILES = HK // 128    # 2
Q_TILES = W // 512      # 2 query tiles per window
S_CHUNKS = W // 128     # 8 key chunks per window


def _build_program(reps=1):
    nc = bacc.Bacc("TRN2", target_bir_lowering=False, debug=False)

    x_d = nc.dram_tensor("x", [WPC * W, C], F32, kind="ExternalInput")
    wq_d = nc.dram_tensor("wq", [C_CHUNKS, 128, HK], BF16, kind="ExternalInput")
    wk_d = nc.dram_tensor("wk", [C_CHUNKS, 128, HK], BF16, kind="ExternalInput")
    wv_d = nc.dram_tensor("wv", [C_CHUNKS, 128, HK], BF16, kind="ExternalInput")
    wo_d = nc.dram_tensor("wo", [HD_TILES, 128, C], BF16, kind="ExternalInput")
    bq_d = nc.dram_tensor("bq", [HD_TILES, 128, 1], F32, kind="ExternalInput")
    bk_d = nc.dram_tensor("bk", [HD_TILES, 128, 1], F32, kind="ExternalInput")
    bo_d = nc.dram_tensor("bo", [1, C], BF16, kind="ExternalInput")
    out_d = nc.dram_tensor("out", [WPC * W, C], F32, kind="ExternalOutput")
    z_d = nc.dram_tensor("z_scratch", [WPC, W, C], BF16)

    with tile.TileContext(nc) as tc:
        with (
            tc.tile_pool(name="const", bufs=1) as const,
            tc.tile_pool(name="xres", bufs=1) as xres,
            tc.tile_pool(name="zt", bufs=1) as ztp,
            tc.tile_pool(name="ln", bufs=4) as ln,
            tc.tile_pool(name="zw", bufs=3) as zw,
            tc.tile_pool(name="qk", bufs=2) as qk,
            tc.tile_pool(name="vp", bufs=2) as vp,
            tc.tile_pool(name="ot", bufs=2) as otp,
            tc.tile_pool(name="ex", bufs=2) as ex,
            tc.tile_pool(name="tmp", bufs=4) as tmp,
            tc.tile_pool(name="outp", bufs=3) as outp,
            tc.tile_pool(name="ps_proj", bufs=2, space="PSUM") as ps_proj,
            tc.tile_pool(name="ps_sc", bufs=1, space="PSUM") as ps_sc_pool,
            tc.tile_pool(name="ps_acc", bufs=1, space="PSUM") as ps_acc,
        ):
            from contextlib import ExitStack as _ES
            _es = _ES()
            if reps > 1:
                _es.enter_context(
                    tc.For_i(
                        0, reps, 1,
                        hint_engines=(
                            mybir.EngineType.PE,
                            mybir.EngineType.Activation,
                            mybir.EngineType.DVE,
                            mybir.EngineType.SP,
                        ),
                    )
                )
            # ---- constants / weights -------------------------------------
            ones32 = const.tile([128, 32], BF16)
            nc.vector.memset(ones32, 1.0)
            ones1 = const.tile([1, 128], BF16)
            nc.vector.memset(ones1, 1.0)
            eps_t = const.tile([128, 1], F32)
            nc.vector.memset(eps_t, EPS)

            wq_s = const.tile([128, C_CHUNKS, HK], BF16)
            wk_s = const.tile([128, C_CHUNKS, HK], BF16)
            wv_s = const.tile([128, C_CHUNKS, HK], BF16)
            for c in range(C_CHUNKS):
                nc.sync.dma_start(wq_s[:, c, :], wq_d[c])
                nc.sync.dma_start(wk_s[:, c, :], wk_d[c])
                nc.sync.dma_start(wv_s[:, c, :], wv_d[c])
            wo_s = const.tile([128, HD_TILES, C], BF16)
            for g in range(HD_TILES):
                nc.sync.dma_start(wo_s[:, g, :], wo_d[g])
            bq_s = const.tile([128, HD_TILES], F32)
            bk_s = const.tile([128, HD_TILES], F32)
            for m in range(HD_TILES):
                nc.sync.dma_start(bq_s[:, m : m + 1], bq_d[m])
                nc.sync.dma_start(bk_s[:, m : m + 1], bk_d[m])
            bo_s = const.tile([1, C], BF16)
            nc.sync.dma_start(bo_s, bo_d[:])

            # ---- phase 1: LayerNorm + transpose bounce (both windows) ----
            xs = [
                [xres.tile([128, C], F32, name=f"x_{w}_{t}", tag=f"x_{w}_{t}") for t in range(TOK_TILES)]
                for w in range(WPC)
            ]
            zT = [
                [ztp.tile([128, W], BF16, name=f"zT_{w}_{c}", tag=f"zT_{w}_{c}") for c in range(C_CHUNKS)]
                for w in range(WPC)
            ]
            for w in range(WPC):
                for t in range(TOK_TILES):
                    x_t = xs[w][t]
                    nc.sync.dma_start(x_t, x_d[(w * TOK_TILES + t) * 128 :][:128, :])
                    stats = ln.tile([128, 6], F32, tag="stats")
                    nc.vector.bn_stats(out=stats, in_=x_t)
                    mv = ln.tile([128, 2], F32, tag="mv")
                    nc.vector.bn_aggr(out=mv, in_=stats)
                    std = ln.tile([128, 1], F32, tag="std")
                    nc.scalar.activation(
                        out=std,
                        in_=mv[:, 1:2],
                        func=mybir.ActivationFunctionType.Sqrt,
                        bias=eps_t[:],
                    )
                    rstd = ln.tile([128, 1], F32, tag="std")
                    nc.vector.reciprocal(out=rstd, in_=std)
                    z_t = zw.tile([128, C], BF16, tag="z")
                    nc.vector.tensor_scalar(
                        out=z_t,
                        in0=x_t,
                        scalar1=mv[:, 0:1],
                        scalar2=rstd,
                        op0=mybir.AluOpType.subtract,
                        op1=mybir.AluOpType.mult,
                    )
                    nc.sync.dma_start(z_d[w, t * 128 :][:128, :], z_t)
                for c in range(C_CHUNKS):
                    nc.sync.dma_start(
                        zT[w][c], z_d[w][:, c * 128 : (c + 1) * 128], transpose=True
                    )

            # ---- phase 2: per-window attention ---------------------------
            for w in range(WPC):
                # --- QKV projections ---
                qkt = {}
                for name, w_s, b_s in (("q", wq_s, bq_s), ("k", wk_s, bk_s)):
                    for m in range(HD_TILES):
                        dst = qk.tile([128, W], BF16, name=f"{name}T_{m}", tag=f"{name}T_{m}")
                        qkt[(name, m)] = dst
                        for n in range(Q_TILES):
                            ps = ps_proj.tile([128, 512], F32, tag="proj")
                            for c in range(C_CHUNKS):
                                nc.tensor.matmul(
                                    ps,
                                    lhsT=w_s[:, c, m * 128 : (m + 1) * 128],
                                    rhs=zT[w][c][:, n * 512 : (n + 1) * 512],
                                    start=(c == 0),
                                    stop=(c == C_CHUNKS - 1),
                                )
                            nc.vector.tensor_scalar_add(
                                out=dst[:, n * 512 : (n + 1) * 512],
                                in0=ps,
                                scalar1=b_s[:, m : m + 1],
                            )
                v_s = vp.tile([128, TOK_TILES, HK], BF16, tag="v")
                for t in range(TOK_TILES):
                    ps = ps_proj.tile([128, 512], F32, tag="proj")
                    psv = ps[:, :HK]
                    for c in range(C_CHUNKS):
                        nc.tensor.matmul(
                            psv,
                            lhsT=zT[w][c][:, t * 128 : (t + 1) * 128],
                            rhs=wv_s[:, c, :],
                            start=(c == 0),
                            stop=(c == C_CHUNKS - 1),
                        )
                    nc.vector.tensor_copy(v_s[:, t, :], psv)

                # --- attention ---
                oT = [otp.tile([128, W], BF16, name=f"oT_{g}", tag=f"oT_{g}") for g in range(HD_TILES)]
                for qt in range(Q_TILES):
                    for hg in range(HD_TILES):
                        ps_oT = ps_acc.tile([128, 512], F32, tag="oT")
                        ps_den = ps_acc.tile([128, 512], F32, tag="den")
                        for cch in range(S_CHUNKS):
                            ps_sc = ps_sc_pool.tile([128, 2048], F32, tag="sc")
                            for g in range(4):
                                nc.tensor.matmul(
                                    ps_sc[:, g * 512 : (g + 1) * 512],
                                    lhsT=qkt[("k", hg)][
                                        g * 32 : (g + 1) * 32,
                                        cch * 128 : (cch + 1) * 128,
                                    ],
                                    rhs=qkt[("q", hg)][
                                        g * 32 : (g + 1) * 32,
                                        qt * 512 : (qt + 1) * 512,
                                    ],
                                    tile_position=(g * 32, 0),
                                )
                            expT = ex.tile([128, 2048], BF16, tag="exp")
                            nc.scalar.activation(
                                out=expT, in_=ps_sc,
                                func=mybir.ActivationFunctionType.Exp,
                                scale=float(SCALE),
                            )
                            for g in range(4):
                                h = 4 * hg + g
                                nc.tensor.matmul(
                                    ps_oT[g * 32 : (g + 1) * 32, :],
                                    lhsT=v_s[:, cch, h * 32 : (h + 1) * 32],
                                    rhs=expT[:, g * 512 : (g + 1) * 512],
                                    start=(cch == 0),
                                    stop=(cch == S_CHUNKS - 1),
                                    tile_position=(0, g * 32),
                                    skip_group_check=True,
                                )
                                nc.tensor.matmul(
                                    ps_den[g * 32 : (g + 1) * 32, :],
                                    lhsT=ones32[:],
                                    rhs=expT[:, g * 512 : (g + 1) * 512],
                                    start=(cch == 0),
                                    stop=(cch == S_CHUNKS - 1),
                                    tile_position=(0, g * 32),
                                    skip_group_check=True,
                                )
                        rec = tmp.tile([128, 512], F32, tag="rec")
                        nc.vector.reciprocal(out=rec, in_=ps_den)
                        nc.vector.tensor_mul(
                            out=oT[hg][:, qt * 512 : (qt + 1) * 512],
                            in0=ps_oT,
                            in1=rec,
                        )

                # --- output projection + residual ---
                for t in range(TOK_TILES):
                    ps = ps_proj.tile([128, 512], F32, tag="proj")
                    for g in range(HD_TILES):
                        nc.tensor.matmul(
                            ps,
                            lhsT=oT[g][:, t * 128 : (t + 1) * 128],
                            rhs=wo_s[:, g, :],
                            start=(g == 0),
                            stop=False,
                        )
                    nc.tensor.matmul(
                        ps, lhsT=ones1[:, :128], rhs=bo_s, start=False, stop=True
                    )
                    o_t = outp.tile([128, C], F32, tag="o")
                    nc.vector.tensor_add(out=o_t, in0=ps, in1=xs[w][t])
                    nc.sync.dma_start(out_d[(w * TOK_TILES + t) * 128 :][:128, :], o_t)

            _es.close()

    nc.compile()
    return nc


_CACHE = {}


def _get_program():
    if "nc" not in _CACHE:
        _CACHE["nc"] = _build_program()
    return _CACHE["nc"]


def _prep_inputs(x, ln_gamma, ln_beta, Wq, bq, Wk, bk, Wv, bv, Wo, bo):
    """Host-side constant folding + sharding. Returns per-core in_maps."""
    x = np.asarray(x, np.float32)
    g = np.asarray(ln_gamma, np.float32)
    be = np.asarray(ln_beta, np.float32)
    Wq = np.asarray(Wq, np.float32).reshape(C, HK)
    Wk = np.asarray(Wk, np.float32).reshape(C, HK)
    Wv = np.asarray(Wv, np.float32).reshape(C, HK)
    Wo2 = np.asarray(Wo, np.float32).reshape(HK, C)
    bq = np.asarray(bq, np.float32).reshape(HK)
    bk = np.asarray(bk, np.float32).reshape(HK)
    bv = np.asarray(bv, np.float32).reshape(HK)
    bo = np.asarray(bo, np.float32).reshape(C)

    # Fold LN affine (z = n*gamma + beta) into projections:
    #   z @ W + b = n @ (gamma[:,None]*W) + (beta @ W + b)
    Wq_e = g[:, None] * Wq
    Wk_e = g[:, None] * Wk
    Wv_e = g[:, None] * Wv
    bq_e = be @ Wq + bq
    bk_e = be @ Wk + bk
    bv_e = be @ Wv + bv
    # Softmax rows sum to 1 -> value bias passes through attention:
    #   attn @ (V + 1 bv) @ Wo + bo = attn @ V @ Wo + (bv @ Wo + bo)
    bo_e = bv_e @ Wo2 + bo

    bf = ml_dtypes.bfloat16
    wq_h = Wq_e.reshape(C_CHUNKS, 128, HK).astype(bf)
    wk_h = Wk_e.reshape(C_CHUNKS, 128, HK).astype(bf)
    wv_h = Wv_e.reshape(C_CHUNKS, 128, HK).astype(bf)
    wo_h = Wo2.reshape(HD_TILES, 128, C).astype(bf)
    bq_h = bq_e.reshape(HD_TILES, 128, 1).astype(np.float32)
    bk_h = bk_e.reshape(HD_TILES, 128, 1).astype(np.float32)
    bo_h = bo_e.reshape(1, C).astype(bf)

    xw = np.ascontiguousarray(x.reshape(NW, W, C))
    in_maps = []
    for i in range(N_CORES):
        shard = np.ascontiguousarray(
            xw[i * WPC : (i + 1) * WPC].reshape(WPC * W, C)
        )
        in_maps.append(
            {
                "x": shard,
                "wq": wq_h, "wk": wk_h, "wv": wv_h, "wo": wo_h,
                "bq": bq_h, "bk": bk_h, "bo": bo_h,
            }
        )
    return in_maps


def kernel(x, ln_gamma, ln_beta, Wq, bq, Wk, bk, Wv, bv, Wo, bo):
    nc = _get_program()
    in_maps = _prep_inputs(x, ln_gamma, ln_beta, Wq, bq, Wk, bk, Wv, bv, Wo, bo)
    res = run_bass_kernel_spmd(nc, in_maps, core_ids=list(range(N_CORES)))
    out = np.concatenate([res.results[i]["out"] for i in range(N_CORES)], axis=0)
    return np.ascontiguousarray(out.reshape(B, T, C)).astype(np.float32)
